# revision 1
# baseline (speedup 1.0000x reference)
"""Boltzformer decoder mask kernel for Trainium2 (8 NeuronCores, SPMD).

Full-input contract: kernel(**inputs) takes the unsharded tensors from
setup_inputs() and returns the full [16, 1024, 1024] float32 output.

Sharding: data-parallel over the B*H=16 leading dim. Core c handles batch
c//4 and the two head-slices (2c, 2c+1). The attention map is identical
across heads within a batch, so each core computes LN -> MLP -> me@me^T ->
sigmoid chain once, and only the rand-dependent tail twice.

Two compiled variants:
- fast: valid when every score is comfortably above the 0.5 attn threshold
  (checked on the host against the actual inputs). Then bp == 0 and
  masked_prob == 1 exactly, so the Boltzmann-normalize chain drops out and
  boltz = sigmoid(100(1-rand)) runs on the otherwise-idle Scalar engine
  during the MLP.
- full: the general computation (used if the guard ever fails).
"""

import math

import numpy as np

B = 2
Q = 1024
D = 256
NUM_HEADS = 8
N_CORES = 8
HEADS_PER_CORE = 2
THRESHOLD = 0.5
N_SAMPLES = int(Q * 0.1)  # 102
LN_EPS = 1e-4
BP_EPS = 1e-6
P = 128  # SBUF partitions
QC = Q // P  # 8 row-chunks per map
FMAP = QC * Q  # [1024,1024] map stored as [128, 8192]

_BUILD_CACHE = {}


def _legalize_waits(nc):
    """TRN2 instruction structs carry only ONE inline sync-wait slot (fp32
    self-loading matmuls, activations, DVE tensor ops, DMA descriptors
    alike). Tile attaches multi-waits; legalize by hoisting the excess waits
    onto standalone same-engine NoOps right before the instruction (the
    raw-bass "wait_ge then op" idiom). Walrus partitions blocks by engine
    preserving order, so a NoOp inserted directly before stays ahead in that
    engine's queue -- semantics are preserved exactly."""
    import concourse.mybir as mybir
    import bass_rust

    skip = ("InstDmaTransposeAnt", "InstTriggerDma")
    for blk in nc.m.functions[0].blocks:
        out_list = []
        for ins in blk.instructions:
            si = getattr(ins, "sync_info", None)
            eng = getattr(ins, "engine", None)
            if (
                si is not None
                and eng is not None
                and type(ins).__name__ not in skip
                and len(si.on_wait) > 1
            ):
                waits = list(si.on_wait)
                for j, w in enumerate(waits[:-1]):
                    nop = mybir.InstNoOp(name=f"{ins.name}-ws{j}", ins=[], outs=[])
                    nop.engine = eng
                    nop.sync_info = bass_rust.SyncInfo(on_wait=[w], on_update=[])
                    out_list.append(nop)
                si.on_wait = [waits[-1]]
            out_list.append(ins)
        blk.instructions = out_list
    return nc


def _build(layer_id: int, fast: bool):
    """Build the per-core Bass program (same NEFF on all 8 cores)."""
    import contextlib

    import concourse.bass as bass
    import concourse.tile as tile
    import concourse.mybir as mybir

    fp32 = mybir.dt.float32
    AF = mybir.ActivationFunctionType
    OP = mybir.AluOpType

    exp_scale = 2.0 + float(layer_id)  # attn / temp == attn * (2 + layer_id)

    nc = bass.Bass("TRN2", target_bir_lowering=False)

    x_d = nc.dram_tensor("x", [Q, D], fp32, kind="ExternalInput")
    w_d = nc.dram_tensor("w", [3, D, D], fp32, kind="ExternalInput")
    b_d = nc.dram_tensor("b", [3, D], fp32, kind="ExternalInput")
    rand_d = nc.dram_tensor("rand", [HEADS_PER_CORE, Q, Q], fp32, kind="ExternalInput")
    out_d = nc.dram_tensor("out", [HEADS_PER_CORE, Q, Q], fp32, kind="ExternalOutput")

    with tile.TileContext(nc) as tc:
        ctx = contextlib.ExitStack()
        with ctx:
            consts = ctx.enter_context(tc.tile_pool(name="consts", bufs=1))
            smalls = ctx.enter_context(tc.tile_pool(name="smalls", bufs=1))
            acts = ctx.enter_context(tc.tile_pool(name="acts", bufs=4))
            maps = ctx.enter_context(
                tc.tile_pool(name="maps", bufs=3 if fast else 5)
            )

            # ---- input DMAs (emission order == DMA priority order) ----
            x_sb = smalls.tile([P, QC, D], fp32)
            x_r = x_d[:, :].rearrange("(t p) d -> p t d", p=P)
            for t in range(QC):
                nc.sync.dma_start(out=x_sb[:, t, :], in_=x_r[:, t, :])
            w_sb = consts.tile([P, 3, 2, D], fp32)
            nc.sync.dma_start(
                out=w_sb, in_=w_d[:, :, :].rearrange("l (kc p) f -> p l kc f", p=P)
            )
            b_sb = consts.tile([P, 3, 2], fp32)
            nc.sync.dma_start(
                out=b_sb, in_=b_d[:, :].rearrange("l (c p) -> p l c", p=P)
            )
            rand_sb = [
                maps.tile([P, FMAP], fp32, tag="maps", name=f"rand_sb{h}")
                for h in range(2)
            ]
            for h in range(2):
                nc.sync.dma_start(
                    out=rand_sb[h].rearrange("p (t k) -> p t k", k=Q),
                    in_=rand_d[h, :, :].rearrange("(t p) k -> p t k", p=P),
                )

            identity = consts.tile([P, P], fp32)
            nc.gpsimd.memset(identity, 0.0)
            nc.gpsimd.affine_select(
                out=identity,
                in_=identity,
                compare_op=OP.not_equal,
                fill=1.0,
                base=0,
                pattern=[[-1, P]],
                channel_multiplier=1,
            )

            # ---- Phase A: LayerNorm (row-major, per 128-row tile) ----
            stats = smalls.tile([P, QC, 6], fp32)
            mv = smalls.tile([P, QC, 2], fp32)
            sd = smalls.tile([P, QC], fp32)
            rstd = smalls.tile([P, QC], fp32)
            eps_t = smalls.tile([P, 1], fp32)
            nc.vector.memset(eps_t, LN_EPS)
            c50_t = smalls.tile([P, 1], fp32)
            nc.vector.memset(c50_t, 50.0)
            c100_t = smalls.tile([P, 1], fp32)
            nc.vector.memset(c100_t, 100.0)
            # xn lives in the acts pool (separate from the DMA'd x tile, so
            # the PE transposes depend only on DVE, not also on the x DMA)
            xn = [
                acts.tile([P, QC // 2, D], fp32, tag="actT", name=f"xn{i}")
                for i in range(2)
            ]
            # fully per-chunk LN pipeline: each 128-row tile flows
            # stats -> sqrt -> recip -> normalize independently
            for t in range(QC):
                nc.vector.bn_stats(out=stats[:, t, :], in_=x_sb[:, t, :])
                nc.vector.bn_aggr(out=mv[:, t, :], in_=stats[:, t, :])
                nc.scalar.activation(
                    out=sd[:, t : t + 1],
                    in_=mv[:, t, 1:2],
                    func=AF.Sqrt,
                    bias=eps_t,
                    scale=1.0,
                )
                nc.vector.reciprocal(
                    out=rstd[:, t : t + 1], in_=sd[:, t : t + 1]
                )
                nc.vector.tensor_scalar(
                    out=xn[t // 4][:, t % 4, :],
                    in0=x_sb[:, t, :],
                    scalar1=mv[:, t, 0:1],
                    scalar2=rstd[:, t : t + 1],
                    op0=OP.subtract,
                    op1=OP.mult,
                )

            # absorb the bias-DMA tick on DVE so MLP bias ops carry <=1 wait
            b_abs = smalls.tile([P, 1], fp32)
            nc.vector.tensor_copy(out=b_abs, in_=b_sb[:, 0, 0:1])

            # ---- Phase B: transpose xn -> xT (feature-major [2][128,1024]) ----
            # fp32 matmuls/transposes are single self-loading instructions with
            # one sync-wait slot: dummy transposes absorb the Pool/DMA ticks,
            # and every PSUM consumer that frees a matmul slot runs on DVE so
            # the slot-WAR and data dep share the DVE semaphore.
            xT = [acts.tile([P, Q], fp32, tag="actT", name=f"xT{h}") for h in range(2)]
            with tc.tile_pool(name="tpsum", bufs=4, space="PSUM") as tpsum, \
                 tc.tile_pool(name="mlpp", bufs=4, space="PSUM") as mlpp:
                for t in range(QC):
                    for h in range(2):
                        pst = tpsum.tile([P, P], fp32)
                        nc.tensor.transpose(
                            pst, xn[t // 4][:, t % 4, h * P : (h + 1) * P], identity
                        )
                        # alternate the PSUM->SBUF copies between the Scalar
                        # and Vector engines: with 4 PSUM slots the copy rate
                        # (not the transpose) paces this chain, so two engines
                        # halve it and the MLP starts ~3us earlier
                        if (t * 2 + h) % 2 == 0:
                            nc.scalar.copy(
                                out=xT[h][:, t * P : (t + 1) * P], in_=pst
                            )
                        else:
                            nc.vector.tensor_copy(
                                out=xT[h][:, t * P : (t + 1) * P], in_=pst
                            )

                if fast:
                    # boltz = sigmoid(100*(1 - rand)), in place over rand, on
                    # the Scalar engine during the MLP (emitted after the
                    # transpose copies so the ACT queue does not block on the
                    # rand DMAs). masked_prob == 1 exactly in this regime.
                    for h in range(2):
                        for qc in range(QC):
                            sl = slice(qc * Q, (qc + 1) * Q)
                            nc.scalar.activation(
                                out=rand_sb[h][:, sl],
                                in_=rand_sb[h][:, sl],
                                func=AF.Sigmoid,
                                scale=-100.0,
                                bias=c100_t,
                            )

                # ---- Phase C: 3-layer MLP in feature-major layout ----
                cur = xT
                for layer in range(3):
                    nxt = [
                        acts.tile([P, Q], fp32, tag="actT", name=f"y{layer}T{f2}")
                        for f2 in range(2)
                    ]
                    for fc in range(2):
                        for rc in range(2):
                            ps = mlpp.tile([P, 512], fp32)
                            for kc in range(2):
                                nc.tensor.matmul(
                                    ps,
                                    lhsT=w_sb[:, layer, kc, fc * P : (fc + 1) * P],
                                    rhs=cur[kc][:, rc * 512 : (rc + 1) * 512],
                                    start=(kc == 0),
                                    stop=(kc == 1),
                                )
                            # bias+relu on DVE (keeps the PSUM slot-freeing
                            # reader on the DVE semaphore for the matmuls)
                            if layer < 2:
                                nc.vector.tensor_scalar(
                                    out=nxt[fc][:, rc * 512 : (rc + 1) * 512],
                                    in0=ps,
                                    scalar1=b_sb[:, layer, fc : fc + 1],
                                    scalar2=0.0,
                                    op0=OP.add,
                                    op1=OP.max,
                                )
                            else:
                                nc.vector.tensor_scalar(
                                    out=nxt[fc][:, rc * 512 : (rc + 1) * 512],
                                    in0=ps,
                                    scalar1=b_sb[:, layer, fc : fc + 1],
                                    scalar2=None,
                                    op0=OP.add,
                                )
                    cur = nxt
            meT = cur  # [2][128, 1024] feature-major me^T

            smask = maps.tile([P, FMAP], fp32, tag="maps")
            out_r = out_d[:, :, :].rearrange("h (t p) k -> h p t k", p=P)
            spsum = ctx.enter_context(
                tc.tile_pool(name="spsum", bufs=4, space="PSUM")
            )

            if fast:
                # ---- fused scores -> attn -> smask -> out, per row-chunk ----
                for qc in range(QC):
                    ps = spsum.tile([P, Q], fp32)
                    for nh in range(2):
                        for kc in range(2):
                            nc.tensor.matmul(
                                ps[:, nh * 512 : (nh + 1) * 512],
                                lhsT=meT[kc][:, qc * P : (qc + 1) * P],
                                rhs=meT[kc][:, nh * 512 : (nh + 1) * 512],
                                start=(kc == 0),
                                stop=(kc == 1),
                            )
                    attn_c = acts.tile([P, Q], fp32, tag="actT", name=f"attn{qc}")
                    # First chunk: its chain gates the start of the saturated
                    # out-DMA stream (span == first-out + total out bytes).
                    # Last chunk: its chain is the kernel tail. Split both
                    # into 512-halves so their stages pipeline.
                    halves = 2 if qc in (0, QC - 1) else 1
                    hw_n = Q // halves
                    for hv in range(halves):
                        c0 = hv * hw_n
                        sl = slice(qc * Q + c0, qc * Q + c0 + hw_n)
                        nc.scalar.activation(
                            out=attn_c[:, c0 : c0 + hw_n],
                            in_=ps[:, c0 : c0 + hw_n],
                            func=AF.Sigmoid,
                            scale=1.0 / math.sqrt(D),
                        )
                        # smask tail, relative-accurate, then quantized exactly
                        # the way the reference's fp32 "1 - sigmoid" rounds:
                        # t<=1e-6 here so sigmoid(-z) == t/(1+t) == (t+1)-1.
                        nc.scalar.activation(
                            out=smask[:, sl],
                            in_=attn_c[:, c0 : c0 + hw_n],
                            func=AF.Exp,
                            scale=-100.0,
                            bias=c50_t,
                        )
                        nc.vector.tensor_scalar(
                            out=smask[:, sl],
                            in0=smask[:, sl],
                            scalar1=1.0,
                            scalar2=1.0,
                            op0=OP.add,
                            op1=OP.subtract,
                        )
                        # out = smask * boltz, in place over the boltz buffer;
                        # head 0 on DVE, head 1 on GpSimd -- except the last
                        # two chunks, where GpSimd would pace the DMA tail
                        for h in range(2):
                            eng = (
                                nc.vector
                                if (h == 0 or qc >= QC - 2)
                                else nc.gpsimd
                            )
                            eng.tensor_tensor(
                                out=rand_sb[h][:, sl],
                                in0=smask[:, sl],
                                in1=rand_sb[h][:, sl],
                                op=OP.mult,
                            )
                            nc.sync.dma_start(
                                out=out_r[h, :, qc, c0 : c0 + hw_n],
                                in_=rand_sb[h][:, sl],
                            )
            else:
                # ---- general path: full Boltzmann chain ----
                chain = maps.tile([P, FMAP], fp32, tag="maps")
                attn = chain
                for qc in range(QC):
                    ps = spsum.tile([P, Q], fp32)
                    for nh in range(2):
                        for kc in range(2):
                            nc.tensor.matmul(
                                ps[:, nh * 512 : (nh + 1) * 512],
                                lhsT=meT[kc][:, qc * P : (qc + 1) * P],
                                rhs=meT[kc][:, nh * 512 : (nh + 1) * 512],
                                start=(kc == 0),
                                stop=(kc == 1),
                            )
                    nc.scalar.activation(
                        out=attn[:, qc * Q : (qc + 1) * Q],
                        in_=ps,
                        func=AF.Sigmoid,
                        scale=1.0 / math.sqrt(D),
                    )

                rs = smalls.tile([P, QC], fp32)
                neg_inv = smalls.tile([P, QC], fp32)
                e_thresh = float(np.exp(np.float32(THRESHOLD * exp_scale)))
                for qc in range(QC):
                    sl = slice(qc * Q, (qc + 1) * Q)
                    # s_mask = 1 - sigmoid((attn-0.5)*100), via the sigmoid LUT
                    # (absolutely accurate; general inputs)
                    nc.scalar.activation(
                        out=smask[:, sl],
                        in_=attn[:, sl],
                        func=AF.Sigmoid,
                        scale=-100.0,
                        bias=c50_t,
                    )
                for qc in range(QC):
                    sl = slice(qc * Q, (qc + 1) * Q)
                    # e2a = exp(attn*scale) in place; threshold compare moves
                    # onto e2a (exp is monotone): attn<0.5 <=> e2a<e^{s/2}
                    nc.scalar.activation(
                        out=chain[:, sl], in_=chain[:, sl], func=AF.Exp,
                        scale=exp_scale,
                    )
                    nc.vector.scalar_tensor_tensor(
                        out=chain[:, sl],
                        in0=chain[:, sl],
                        scalar=e_thresh,
                        in1=chain[:, sl],
                        op0=OP.is_lt,
                        op1=OP.mult,
                        accum_out=rs[:, qc : qc + 1],
                    )
                nc.vector.tensor_scalar(
                    out=neg_inv,
                    in0=rs,
                    scalar1=-1.0,
                    scalar2=-BP_EPS,
                    op0=OP.mult,
                    op1=OP.add,
                )
                nc.vector.reciprocal(out=neg_inv, in_=neg_inv)
                for qc in range(QC):
                    sl = slice(qc * Q, (qc + 1) * Q)
                    nc.scalar.activation(
                        out=chain[:, sl],
                        in_=chain[:, sl],
                        func=AF.Ln,
                        scale=neg_inv[:, qc : qc + 1],
                        bias=1.0,
                    )
                    nc.scalar.activation(
                        out=chain[:, sl],
                        in_=chain[:, sl],
                        func=AF.Exp,
                        scale=float(N_SAMPLES),
                    )
                mp = chain

                dve_abs = smalls.tile([P, 2], fp32)
                pool_abs = smalls.tile([P, 2], fp32)
                nc.vector.tensor_copy(out=dve_abs[:, 0:1], in_=rand_sb[0][:, 0:1])
                nc.vector.tensor_copy(out=dve_abs[:, 1:2], in_=rand_sb[1][:, 0:1])
                nc.gpsimd.tensor_copy(out=pool_abs[:, 0:1], in_=rand_sb[0][:, 0:1])
                nc.gpsimd.tensor_copy(out=pool_abs[:, 1:2], in_=rand_sb[1][:, 0:1])

                work = [
                    maps.tile([P, FMAP], fp32, tag="maps", name=f"work{h}")
                    for h in range(2)
                ]
                for h in range(2):
                    sub_eng = nc.vector if h == 0 else nc.gpsimd
                    for qc in range(QC):
                        sl = slice(qc * Q, (qc + 1) * Q)
                        sub_eng.tensor_tensor(
                            out=work[h][:, sl],
                            in0=mp[:, sl],
                            in1=rand_sb[h][:, sl],
                            op=OP.subtract,
                        )
                        nc.scalar.activation(
                            out=rand_sb[h][:, sl],
                            in_=work[h][:, sl],
                            func=AF.Sigmoid,
                            scale=100.0,
                        )
                        mul_eng = nc.vector if h == 0 else nc.gpsimd
                        mul_eng.tensor_tensor(
                            out=work[h][:, sl],
                            in0=smask[:, sl],
                            in1=rand_sb[h][:, sl],
                            op=OP.mult,
                        )
                        nc.sync.dma_start(
                            out=out_r[h, :, qc, :],
                            in_=work[h][:, sl],
                        )

    return _legalize_waits(nc)


def _get_nc(layer_id: int, fast: bool):
    key = (int(layer_id), bool(fast))
    if key not in _BUILD_CACHE:
        _BUILD_CACHE[key] = _build(*key)
    return _BUILD_CACHE[key]


def _fast_path_ok(tgt_mask, w_all, b_all, layer_id):
    """Host-side guard: the fast kernel assumes every attn value stays above
    the 0.5 threshold with margin (so bp==0, masked_prob==1, and the smask
    tail stays < 1e-2). Verify on the actual inputs with a cheap numpy pass.
    (layer_id only scales the Boltzmann exponent, which is inert when bp==0,
    so it does not affect fast-path validity.)"""
    del layer_id
    x = tgt_mask.astype(np.float32)
    mu = x.mean(-1, keepdims=True)
    var = x.var(-1, keepdims=True)
    xn = (x - mu) / np.sqrt(var + LN_EPS)
    h = np.maximum(xn @ w_all[0] + b_all[0], 0.0)
    h = np.maximum(h @ w_all[1] + b_all[1], 0.0)
    me = h @ w_all[2] + b_all[2]
    zmin = np.inf
    for b in range(me.shape[0]):
        s = (me[b] @ me[b].T) / np.float32(math.sqrt(D))
        zmin = min(zmin, float(s.min()))
    return zmin > 0.25


def _run(
    tgt_mask,
    ln_w,
    ln_b,
    w1,
    b1,
    w2,
    b2,
    w3,
    b3,
    rand,
    layer_id,
    trace=False,
    force_path=None,
):
    from concourse.bass_utils import run_bass_kernel_spmd

    tgt_mask = np.asarray(tgt_mask, np.float32)
    ln_w = np.asarray(ln_w, np.float32)
    ln_b = np.asarray(ln_b, np.float32)
    w1 = np.asarray(w1, np.float32)
    b1 = np.asarray(b1, np.float32)
    w2 = np.asarray(w2, np.float32)
    b2 = np.asarray(b2, np.float32)
    w3 = np.asarray(w3, np.float32)
    b3 = np.asarray(b3, np.float32)
    rand = np.asarray(rand, np.float32)
    lid = int(np.asarray(layer_id))

    # Fold the layernorm affine params into layer 1: LN(x)*g + c then @w1+b1
    # == LN(x) @ (g[:,None]*w1) + (c@w1 + b1).
    w1f = (ln_w[:, None] * w1).astype(np.float32)
    b1f = (ln_b @ w1 + b1).astype(np.float32)
    w_all = np.ascontiguousarray(np.stack([w1f, w2, w3]), np.float32)
    b_all = np.ascontiguousarray(np.stack([b1f, b2, b3]), np.float32)

    if force_path is None:
        fast = _fast_path_ok(tgt_mask, w_all, b_all, lid)
    else:
        fast = force_path == "fast"
    nc = _get_nc(lid, fast)

    in_maps = []
    for c in range(N_CORES):
        b = c // (N_CORES // B)
        in_maps.append(
            {
                "x": np.ascontiguousarray(tgt_mask[b]),
                "w": w_all,
                "b": b_all,
                "rand": np.ascontiguousarray(
                    rand[c * HEADS_PER_CORE : (c + 1) * HEADS_PER_CORE]
                ),
            }
        )

    res = run_bass_kernel_spmd(
        nc, in_maps, core_ids=list(range(N_CORES)), trace=trace
    )
    out = np.concatenate([res.results[c]["out"] for c in range(N_CORES)], axis=0)
    return out.astype(np.float32), res


def kernel(**inputs):
    out, _ = _run(**inputs)
    return out



# revision 27
# speedup vs baseline: 1.0420x; 1.0420x over previous
"""Boltzformer decoder mask kernel for Trainium2 (8 NeuronCores, SPMD).

Full-input contract: kernel(**inputs) takes the unsharded tensors from
setup_inputs() and returns the full [16, 1024, 1024] float32 output.

Sharding: data-parallel over the B*H=16 leading dim. Core c handles batch
c//4 and the two head-slices (2c, 2c+1). The attention map is identical
across heads within a batch, so each core computes LN -> MLP -> me@me^T ->
sigmoid chain once, and only the rand-dependent tail twice.

Two compiled variants:
- fast: valid when every score is comfortably above the 0.5 attn threshold
  (checked on the host against the actual inputs). Then bp == 0 and
  masked_prob == 1 exactly, so the Boltzmann-normalize chain drops out.
  This variant trades bit-exactness for speed within the 2e-2 gate:
  matmuls run in float32r (bf16x2-precision PE fast path, ~2^-16 rel),
  rand streams in as fp16 (host-rounded; boltz sees <=2.4e-4 abs noise),
  and smask/boltz/out are bf16 (fp32 exponent range -- smask ~1e-6 must
  not hit fp16 subnormals). Net output noise ~0.3% rms vs the 2% gate.
- full: the general computation, bit-conservative fp32 throughout (used
  if the guard ever fails).
"""

import math

import numpy as np

B = 2
Q = 1024
D = 256
NUM_HEADS = 8
N_CORES = 8
HEADS_PER_CORE = 2
THRESHOLD = 0.5
N_SAMPLES = int(Q * 0.1)  # 102
LN_EPS = 1e-4
BP_EPS = 1e-6
P = 128  # SBUF partitions
QC = Q // P  # 8 row-chunks per map
FMAP = QC * Q  # [1024,1024] map stored as [128, 8192]

_BUILD_CACHE = {}


def _legalize_waits(nc):
    """TRN2 instruction structs carry only ONE inline sync-wait slot (fp32
    self-loading matmuls, activations, DVE tensor ops, DMA descriptors
    alike). Tile attaches multi-waits; legalize by hoisting the excess waits
    onto standalone same-engine NoOps right before the instruction (the
    raw-bass "wait_ge then op" idiom). Walrus partitions blocks by engine
    preserving order, so a NoOp inserted directly before stays ahead in that
    engine's queue -- semantics are preserved exactly."""
    import concourse.mybir as mybir
    import bass_rust

    skip = ("InstDmaTransposeAnt", "InstTriggerDma")
    for blk in nc.m.functions[0].blocks:
        out_list = []
        for ins in blk.instructions:
            si = getattr(ins, "sync_info", None)
            eng = getattr(ins, "engine", None)
            if (
                si is not None
                and eng is not None
                and type(ins).__name__ not in skip
                and len(si.on_wait) > 1
            ):
                waits = list(si.on_wait)
                for j, w in enumerate(waits[:-1]):
                    nop = mybir.InstNoOp(name=f"{ins.name}-ws{j}", ins=[], outs=[])
                    nop.engine = eng
                    nop.sync_info = bass_rust.SyncInfo(on_wait=[w], on_update=[])
                    out_list.append(nop)
                si.on_wait = [waits[-1]]
            out_list.append(ins)
        blk.instructions = out_list
    return nc


def _build_fast(nc, tile, mybir):
    """Fast-path per-core program.

    Engine budget per core (cost-model): DMA 10.2 MiB ~= 28.7us pooled,
    ACT is the pacing engine. ACT work: 2 boltz map passes (8x 2-chunk
    sigmoid ops, 15.1us) + attn-sigmoid/exp restricted to the UPPER
    TRIANGLE of the symmetric attention map (scores = me@me^T, so
    smask[q,k] == smask[k,q]; lower-triangle blocks are mirrored with
    cheap PE 128x128 transposes instead of ACT passes) -- ~11us instead
    of 17.4us. PE: fp32r matmuls (MLP + triangular scores + mirrors)
    ~12us. Mults on DVE (bf16 2x mode). The DMA order interleaves x /
    rand / weights so ACT starts ~5.4us in and meT lands ~15us.
    """
    import contextlib

    fp32 = mybir.dt.float32
    fp32r = mybir.dt.float32r
    fp16 = mybir.dt.float16
    bf16 = mybir.dt.bfloat16
    AF = mybir.ActivationFunctionType
    OP = mybir.AluOpType

    x_d = nc.dram_tensor("x", [Q, D], fp32, kind="ExternalInput")
    w_d = nc.dram_tensor("w", [3, D, D], fp32, kind="ExternalInput")
    b_d = nc.dram_tensor("b", [3, D], fp32, kind="ExternalInput")
    rand_d = nc.dram_tensor("rand", [HEADS_PER_CORE, Q, Q], fp16, kind="ExternalInput")
    out_d = nc.dram_tensor("out", [HEADS_PER_CORE, Q, Q], bf16, kind="ExternalOutput")

    with tile.TileContext(nc) as tc:
        ctx = contextlib.ExitStack()
        with ctx:
            consts = ctx.enter_context(tc.tile_pool(name="consts", bufs=1))
            smalls = ctx.enter_context(tc.tile_pool(name="smalls", bufs=1))
            xnp = ctx.enter_context(tc.tile_pool(name="xnp", bufs=2))
            acts = ctx.enter_context(tc.tile_pool(name="acts", bufs=8))
            maps = ctx.enter_context(tc.tile_pool(name="maps", bufs=5))
            attnp = ctx.enter_context(tc.tile_pool(name="attnp", bufs=4))
            outp = ctx.enter_context(tc.tile_pool(name="outp", bufs=4))

            # ---- input DMAs (emission order == DMA priority order) ----
            # Interleave x / rand / w: x gates LN->MLP (needs to finish by
            # ~13us), the first rand slices gate ACT's boltz start (~5.4us),
            # w gates the MLP (~9.7us).
            x_sb = smalls.tile([P, QC, D], fp32)
            x_r = x_d[:, :].rearrange("(t p) d -> p t d", p=P)

            def x_dma(g):  # 2-chunk slice of x
                nc.sync.dma_start(
                    out=x_sb[:, 2 * g : 2 * g + 2, :],
                    in_=x_r[:, 2 * g : 2 * g + 2, :],
                )

            rand_sb = [
                maps.tile([P, FMAP], fp16, tag="maps", name=f"rand_sb{h}")
                for h in range(2)
            ]
            rand_r = [
                rand_d[h, :, :].rearrange("(t p) k -> p t k", p=P) for h in range(2)
            ]

            def rand_dma(h, g):  # 2-chunk slice [P, 2, 1024] of head h
                nc.sync.dma_start(
                    out=rand_sb[h].rearrange("p (t k) -> p t k", k=Q)[
                        :, 2 * g : 2 * g + 2, :
                    ],
                    in_=rand_r[h][:, 2 * g : 2 * g + 2, :],
                )

            x_dma(0)
            rand_dma(0, 0)
            x_dma(1)
            rand_dma(1, 0)
            x_dma(2)
            x_dma(3)
            w_sb = consts.tile([P, 3, 2, D], fp32)
            nc.sync.dma_start(
                out=w_sb, in_=w_d[:, :, :].rearrange("l (kc p) f -> p l kc f", p=P)
            )
            b_sb = consts.tile([P, 3, 2], fp32)
            nc.sync.dma_start(
                out=b_sb, in_=b_d[:, :].rearrange("l (c p) -> p l c", p=P)
            )
            rand_dma(0, 1)
            rand_dma(1, 1)
            rand_dma(0, 2)
            rand_dma(1, 2)
            rand_dma(0, 3)
            rand_dma(1, 3)

            identity = consts.tile([P, P], fp32)
            nc.gpsimd.memset(identity, 0.0)
            nc.gpsimd.affine_select(
                out=identity,
                in_=identity,
                compare_op=OP.not_equal,
                fill=1.0,
                base=0,
                pattern=[[-1, P]],
                channel_multiplier=1,
            )
            id_bf = consts.tile([P, P], bf16)
            nc.gpsimd.memset(id_bf, 0.0)
            nc.gpsimd.affine_select(
                out=id_bf,
                in_=id_bf,
                compare_op=OP.not_equal,
                fill=1.0,
                base=0,
                pattern=[[-1, P]],
                channel_multiplier=1,
            )

            # ---- small constants ----
            stats = smalls.tile([P, QC, 6], fp32)
            mv = smalls.tile([P, QC, 2], fp32)
            sd = smalls.tile([P, QC], fp32)
            rstd = smalls.tile([P, QC], fp32)
            eps_t = smalls.tile([P, 1], fp32)
            nc.vector.memset(eps_t, LN_EPS)
            c50_t = smalls.tile([P, 1], fp32)
            nc.vector.memset(c50_t, 50.0)
            c100_t = smalls.tile([P, 1], fp32)
            nc.vector.memset(c100_t, 100.0)

            # maps-sized tiles: rand fp16 x2, boltz bf16 x2, smask bf16
            boltz = [
                maps.tile([P, FMAP], bf16, tag="maps", name=f"boltz{h}")
                for h in range(2)
            ]
            smask = maps.tile([P, FMAP], bf16, tag="maps", name="smask")

            def boltz_op(h, g):  # one 2-chunk sigmoid over rand slice (h, g)
                sl = slice(2 * g * Q, (2 * g + 2) * Q)
                nc.scalar.activation(
                    out=boltz[h][:, sl],
                    in_=rand_sb[h][:, sl],
                    func=AF.Sigmoid,
                    scale=-100.0,
                    bias=c100_t,
                )

            def sq_op(g):  # batched sqrt(var+eps) for chunks 2g, 2g+1 on ACT
                nc.scalar.activation(
                    out=sd[:, 2 * g : 2 * g + 2],
                    in_=mv[:, 2 * g : 2 * g + 2, 1],
                    func=AF.Sqrt,
                    bias=eps_t,
                    scale=1.0,
                )

            # ---- LayerNorm: DVE stats in x-arrival order; ACT sqrts are
            # batched per 2 chunks and interleaved between boltz ops below.
            xn = [
                xnp.tile([P, QC // 2, D], fp32, tag="xn", name=f"xn{i}")
                for i in range(2)
            ]

            def ln_stats(t):
                nc.vector.bn_stats(out=stats[:, t, :], in_=x_sb[:, t, :])
                nc.vector.bn_aggr(out=mv[:, t, :], in_=stats[:, t, :])

            def ln_xn(t):
                nc.vector.reciprocal(
                    out=rstd[:, t : t + 1], in_=sd[:, t : t + 1]
                )
                nc.vector.tensor_scalar(
                    out=xn[t // 4][:, t % 4, :],
                    in0=x_sb[:, t, :],
                    scalar1=mv[:, t, 0:1],
                    scalar2=rstd[:, t : t + 1],
                    op0=OP.subtract,
                    op1=OP.mult,
                )

            for t in range(4):
                ln_stats(t)
            # ACT queue head: first boltz slice, then the batched LN sqrts
            # (so they never head-block the boltz stream), second boltz.
            boltz_op(0, 0)
            sq_op(0)
            sq_op(1)
            for t in range(4, 8):
                ln_stats(t)
            for t in range(4):
                ln_xn(t)
            boltz_op(1, 0)
            with tc.high_priority():
                sq_op(2)
                sq_op(3)
                for t in range(4, 8):
                    ln_xn(t)

            xT = [
                acts.tile([P, Q], fp32, tag="actT", name=f"xT{h}")
                for h in range(2)
            ]
            with tc.tile_pool(name="tpsum", bufs=4, space="PSUM") as tpsum, \
                 tc.tile_pool(name="mlpp", bufs=4, space="PSUM") as mlpp:

                def trans_chunk(t):  # xn chunk t -> xT columns, both halves
                    for h in range(2):
                        pst = tpsum.tile([P, P], fp32)
                        nc.tensor.transpose(
                            pst, xn[t // 4][:, t % 4, h * P : (h + 1) * P], identity
                        )
                        if t < 4:
                            nc.vector.tensor_copy(
                                out=xT[h][:, t * P : (t + 1) * P], in_=pst
                            )
                        else:
                            nc.scalar.copy(
                                out=xT[h][:, t * P : (t + 1) * P], in_=pst
                            )

                # ---- MLP: float32r; emitted per (layer, token-block) so
                # the PE queue reaches layer L+1 of token-block 0 without
                # head-blocking on late chunk-4..7 transposes. Biases: fc0
                # on DVE, fc1 on Pool.
                y = [
                    [
                        acts.tile([P, Q], fp32, tag="actT", name=f"y{layer}T{f2}")
                        for f2 in range(2)
                    ]
                    for layer in range(3)
                ]

                def mlp_lrc(layer, rc):
                    cur = xT if layer == 0 else y[layer - 1]
                    if True:
                        for fc in range(2):
                            ps = mlpp.tile([P, 512], fp32)
                            for kc in range(2):
                                nc.tensor.matmul(
                                    ps,
                                    lhsT=w_sb[
                                        :, layer, kc, fc * P : (fc + 1) * P
                                    ],
                                    rhs=cur[kc][
                                        :, rc * 512 : (rc + 1) * 512
                                    ],
                                    start=(kc == 0),
                                    stop=(kc == 1),
                                )
                            if layer == 0 and rc == 0:
                                nc.scalar.activation(
                                    out=y[0][fc][:, 0:512],
                                    in_=ps,
                                    func=AF.Relu,
                                    bias=b_sb[:, 0, fc : fc + 1],
                                    scale=1.0,
                                )
                                continue
                            eng = nc.vector
                            if layer < 2:
                                eng.tensor_scalar(
                                    out=y[layer][fc][:, rc * 512 : (rc + 1) * 512],
                                    in0=ps,
                                    scalar1=b_sb[:, layer, fc : fc + 1],
                                    scalar2=0.0,
                                    op0=OP.add,
                                    op1=OP.max,
                                )
                            else:
                                eng.tensor_scalar(
                                    out=y[layer][fc][:, rc * 512 : (rc + 1) * 512],
                                    in0=ps,
                                    scalar1=b_sb[:, layer, fc : fc + 1],
                                    scalar2=None,
                                    op0=OP.add,
                                )

                for t in range(4):
                    trans_chunk(t)
                mlp_lrc(0, 0)
                for t in range(4, 8):
                    trans_chunk(t)
                mlp_lrc(0, 1)
                mlp_lrc(1, 0)
                mlp_lrc(1, 1)
                mlp_lrc(2, 0)
                mlp_lrc(2, 1)
            meT = y[2]  # [2][128, 1024] feature-major me^T

            out_r = out_d[:, :, :].rearrange("h (t p) k -> p h t k", p=P)
            spsum = ctx.enter_context(
                tc.tile_pool(name="spsum", bufs=3, space="PSUM")
            )
            mirp = ctx.enter_context(
                tc.tile_pool(name="mirp", bufs=2, space="PSUM")
            )

            # ---- triangular scores -> attn -> smask (upper bands only) ----
            # Band qc covers columns [qc*128, 1024). Piece boundaries are
            # aligned to 512 so piece 1 only needs meT token-block 0.
            band_ps = {}
            attn_bands = {}

            def band_pieces(qc):
                lo = qc * P
                if lo < 512:
                    return [(lo, 512), (512, Q)]
                return [(lo, Q)]

            def scores_band(qc):
                # ps spans the full row so each piece's matmul output stays
                # inside one 2KB PSUM bank (a matmul output must not cross
                # a bank boundary; only cols [qc*128, 1024) are written).
                lo = qc * P
                ps = spsum.tile([P, Q], fp32, tag="sps")
                band_ps[qc] = ps
                attn_bands[qc] = attnp.tile(
                    [P, Q], fp32, tag="attn", name=f"attn{qc}"
                )
                for c0, c1 in band_pieces(qc):
                    for kc in range(2):
                        nc.tensor.matmul(
                            ps[:, c0:c1],
                            lhsT=meT[kc][:, lo : lo + P],
                            rhs=meT[kc][:, c0:c1],
                            start=(kc == 0),
                            stop=(kc == 1),
                        )

            def sig_exp(qc, piece=None):
                # sigmoid (PSUM->attn fp32) then exp -> smask band, bf16.
                # Exp LUT keeps RELATIVE accuracy at the ~1e-6 magnitudes
                # that dominate the output norm (sigmoid LUT would not).
                lo = qc * P
                pieces = [(lo, Q)] if piece is None else [piece]
                ps = band_ps[qc]
                attn_c = attn_bands[qc]
                for c0, c1 in pieces:
                    nc.scalar.activation(
                        out=attn_c[:, c0:c1],
                        in_=ps[:, c0:c1],
                        func=AF.Sigmoid,
                        scale=1.0 / math.sqrt(D),
                    )
                    # t = exp(-100*(attn-0.5)), in place (fp32)
                    nc.scalar.activation(
                        out=attn_c[:, c0:c1],
                        in_=attn_c[:, c0:c1],
                        func=AF.Exp,
                        scale=-100.0,
                        bias=c50_t,
                    )
                    # smask = (t+1)-1 in fp32, rounded to bf16 on write:
                    # reproduces the reference's catastrophic-cancellation
                    # quantization of 1-sigmoid(z) (multiples of 2^-23 --
                    # 8.4% of the output norm IS this quantization), and
                    # those grid values are exact in bf16. Also snaps away
                    # any hardware Exp-LUT noise below half a grid step.
                    qeng = nc.vector if qc < 4 else nc.gpsimd
                    qeng.tensor_scalar(
                        out=smask[:, qc * Q + c0 : qc * Q + c1],
                        in0=attn_c[:, c0:c1],
                        scalar1=1.0,
                        scalar2=1.0,
                        op0=OP.add,
                        op1=OP.subtract,
                    )

            def mirror_group(b):
                # reflect band b's blocks (a, b) for a > b: PE transpose of
                # each bf16 [128,128] block into its own PSUM tile (4 slots
                # cycle), then PSUM->SBUF copy. Ascending a: the nearest row
                # (needed soonest by mult_out) is mirrored first.
                for a in range(b + 1, QC):
                    pst = mirp.tile([P, P], bf16, tag="mir")
                    nc.tensor.transpose(
                        pst, smask[:, b * Q + a * P : b * Q + (a + 1) * P], id_bf
                    )
                    nc.vector.tensor_copy(
                        out=smask[:, a * Q + b * P : a * Q + (b + 1) * P],
                        in_=pst,
                    )

            out_tiles = {}

            def mult_out(qc, piece=None):
                if qc not in out_tiles:
                    out_tiles[qc] = outp.tile(
                        [P, 2, Q], bf16, tag="out", name=f"out{qc}"
                    )
                out_t = out_tiles[qc]
                c0, c1 = piece if piece is not None else (0, Q)
                for h in range(2):
                    eng = nc.gpsimd if (h == 1 and qc < 6) else nc.vector
                    eng.tensor_tensor(
                        out=out_t[:, h, c0:c1],
                        in0=smask[:, qc * Q + c0 : qc * Q + c1],
                        in1=boltz[h][:, qc * Q + c0 : qc * Q + c1],
                        op=OP.mult,
                    )
                nc.sync.dma_start(
                    out=out_r[:, :, qc, c0:c1],
                    in_=out_t[:, :, c0:c1],
                )

            # ---- tail: the interleave order below IS each engine's queue
            # order (ACT: boltz/sigmoid/exp; PE: scores then mirrors; DVE:
            # mirror copies + mults). Tuned so ACT never head-blocks and the
            # out-DMA stream starts as early as possible.
            scores_band(0)
            scores_band(1)
            boltz_op(0, 1)
            boltz_op(1, 1)
            scores_band(2)
            sig_exp(0, piece=(0, 512))
            sig_exp(0, piece=(512, Q))
            boltz_op(0, 2)
            scores_band(3)
            mirror_group(0)
            mult_out(0)
            sig_exp(1)
            mult_out(1)
            scores_band(4)
            mirror_group(1)
            boltz_op(1, 2)
            sig_exp(2)
            mult_out(2)
            scores_band(5)
            mirror_group(2)
            sig_exp(3)
            mult_out(3)
            boltz_op(0, 3)
            scores_band(6)
            mirror_group(3)
            sig_exp(4)
            mult_out(4)
            boltz_op(1, 3)
            scores_band(7)
            mirror_group(4)
            sig_exp(5)
            mult_out(5)
            mult_out(6, piece=(0, 512))
            mult_out(7, piece=(0, 512))
            mirror_group(5)
            sig_exp(6)
            mirror_group(6)
            sig_exp(7)
            mult_out(6, piece=(512, Q))
            mult_out(7, piece=(512, Q))

    return nc


def _build_full(nc, tile, mybir, layer_id):
    """General path: full Boltzmann chain, conservative fp32 (bit-matched
    to the reference where possible). Same code as the original baseline."""
    import contextlib

    fp32 = mybir.dt.float32
    AF = mybir.ActivationFunctionType
    OP = mybir.AluOpType

    exp_scale = 2.0 + float(layer_id)  # attn / temp == attn * (2 + layer_id)

    x_d = nc.dram_tensor("x", [Q, D], fp32, kind="ExternalInput")
    w_d = nc.dram_tensor("w", [3, D, D], fp32, kind="ExternalInput")
    b_d = nc.dram_tensor("b", [3, D], fp32, kind="ExternalInput")
    rand_d = nc.dram_tensor("rand", [HEADS_PER_CORE, Q, Q], fp32, kind="ExternalInput")
    out_d = nc.dram_tensor("out", [HEADS_PER_CORE, Q, Q], fp32, kind="ExternalOutput")

    with tile.TileContext(nc) as tc:
        ctx = contextlib.ExitStack()
        with ctx:
            consts = ctx.enter_context(tc.tile_pool(name="consts", bufs=1))
            smalls = ctx.enter_context(tc.tile_pool(name="smalls", bufs=1))
            acts = ctx.enter_context(tc.tile_pool(name="acts", bufs=4))
            maps = ctx.enter_context(tc.tile_pool(name="maps", bufs=5))

            x_sb = smalls.tile([P, QC, D], fp32)
            x_r = x_d[:, :].rearrange("(t p) d -> p t d", p=P)
            for t in range(QC):
                nc.sync.dma_start(out=x_sb[:, t, :], in_=x_r[:, t, :])
            w_sb = consts.tile([P, 3, 2, D], fp32)
            nc.sync.dma_start(
                out=w_sb, in_=w_d[:, :, :].rearrange("l (kc p) f -> p l kc f", p=P)
            )
            b_sb = consts.tile([P, 3, 2], fp32)
            nc.sync.dma_start(
                out=b_sb, in_=b_d[:, :].rearrange("l (c p) -> p l c", p=P)
            )
            rand_sb = [
                maps.tile([P, FMAP], fp32, tag="maps", name=f"rand_sb{h}")
                for h in range(2)
            ]
            for h in range(2):
                nc.sync.dma_start(
                    out=rand_sb[h].rearrange("p (t k) -> p t k", k=Q),
                    in_=rand_d[h, :, :].rearrange("(t p) k -> p t k", p=P),
                )

            identity = consts.tile([P, P], fp32)
            nc.gpsimd.memset(identity, 0.0)
            nc.gpsimd.affine_select(
                out=identity,
                in_=identity,
                compare_op=OP.not_equal,
                fill=1.0,
                base=0,
                pattern=[[-1, P]],
                channel_multiplier=1,
            )

            stats = smalls.tile([P, QC, 6], fp32)
            mv = smalls.tile([P, QC, 2], fp32)
            sd = smalls.tile([P, QC], fp32)
            rstd = smalls.tile([P, QC], fp32)
            eps_t = smalls.tile([P, 1], fp32)
            nc.vector.memset(eps_t, LN_EPS)
            c50_t = smalls.tile([P, 1], fp32)
            nc.vector.memset(c50_t, 50.0)
            c100_t = smalls.tile([P, 1], fp32)
            nc.vector.memset(c100_t, 100.0)
            xn = [
                acts.tile([P, QC // 2, D], fp32, tag="actT", name=f"xn{i}")
                for i in range(2)
            ]
            for t in range(QC):
                nc.vector.bn_stats(out=stats[:, t, :], in_=x_sb[:, t, :])
                nc.vector.bn_aggr(out=mv[:, t, :], in_=stats[:, t, :])
                nc.scalar.activation(
                    out=sd[:, t : t + 1],
                    in_=mv[:, t, 1:2],
                    func=AF.Sqrt,
                    bias=eps_t,
                    scale=1.0,
                )
                nc.vector.reciprocal(
                    out=rstd[:, t : t + 1], in_=sd[:, t : t + 1]
                )
                nc.vector.tensor_scalar(
                    out=xn[t // 4][:, t % 4, :],
                    in0=x_sb[:, t, :],
                    scalar1=mv[:, t, 0:1],
                    scalar2=rstd[:, t : t + 1],
                    op0=OP.subtract,
                    op1=OP.mult,
                )

            b_abs = smalls.tile([P, 1], fp32)
            nc.vector.tensor_copy(out=b_abs, in_=b_sb[:, 0, 0:1])

            xT = [acts.tile([P, Q], fp32, tag="actT", name=f"xT{h}") for h in range(2)]
            with tc.tile_pool(name="tpsum", bufs=4, space="PSUM") as tpsum, \
                 tc.tile_pool(name="mlpp", bufs=4, space="PSUM") as mlpp:
                for t in range(QC):
                    for h in range(2):
                        pst = tpsum.tile([P, P], fp32)
                        nc.tensor.transpose(
                            pst, xn[t // 4][:, t % 4, h * P : (h + 1) * P], identity
                        )
                        if (t * 2 + h) % 2 == 0:
                            nc.scalar.copy(
                                out=xT[h][:, t * P : (t + 1) * P], in_=pst
                            )
                        else:
                            nc.vector.tensor_copy(
                                out=xT[h][:, t * P : (t + 1) * P], in_=pst
                            )

                cur = xT
                for layer in range(3):
                    nxt = [
                        acts.tile([P, Q], fp32, tag="actT", name=f"y{layer}T{f2}")
                        for f2 in range(2)
                    ]
                    for fc in range(2):
                        for rc in range(2):
                            ps = mlpp.tile([P, 512], fp32)
                            for kc in range(2):
                                nc.tensor.matmul(
                                    ps,
                                    lhsT=w_sb[:, layer, kc, fc * P : (fc + 1) * P],
                                    rhs=cur[kc][:, rc * 512 : (rc + 1) * 512],
                                    start=(kc == 0),
                                    stop=(kc == 1),
                                )
                            if layer < 2:
                                nc.vector.tensor_scalar(
                                    out=nxt[fc][:, rc * 512 : (rc + 1) * 512],
                                    in0=ps,
                                    scalar1=b_sb[:, layer, fc : fc + 1],
                                    scalar2=0.0,
                                    op0=OP.add,
                                    op1=OP.max,
                                )
                            else:
                                nc.vector.tensor_scalar(
                                    out=nxt[fc][:, rc * 512 : (rc + 1) * 512],
                                    in0=ps,
                                    scalar1=b_sb[:, layer, fc : fc + 1],
                                    scalar2=None,
                                    op0=OP.add,
                                )
                    cur = nxt
            meT = cur

            smask = maps.tile([P, FMAP], fp32, tag="maps")
            out_r = out_d[:, :, :].rearrange("h (t p) k -> h p t k", p=P)
            spsum = ctx.enter_context(
                tc.tile_pool(name="spsum", bufs=4, space="PSUM")
            )

            chain = maps.tile([P, FMAP], fp32, tag="maps")
            attn = chain
            for qc in range(QC):
                ps = spsum.tile([P, Q], fp32)
                for nh in range(2):
                    for kc in range(2):
                        nc.tensor.matmul(
                            ps[:, nh * 512 : (nh + 1) * 512],
                            lhsT=meT[kc][:, qc * P : (qc + 1) * P],
                            rhs=meT[kc][:, nh * 512 : (nh + 1) * 512],
                            start=(kc == 0),
                            stop=(kc == 1),
                        )
                nc.scalar.activation(
                    out=attn[:, qc * Q : (qc + 1) * Q],
                    in_=ps,
                    func=AF.Sigmoid,
                    scale=1.0 / math.sqrt(D),
                )

            rs = smalls.tile([P, QC], fp32)
            neg_inv = smalls.tile([P, QC], fp32)
            e_thresh = float(np.exp(np.float32(THRESHOLD * exp_scale)))
            for qc in range(QC):
                sl = slice(qc * Q, (qc + 1) * Q)
                nc.scalar.activation(
                    out=smask[:, sl],
                    in_=attn[:, sl],
                    func=AF.Sigmoid,
                    scale=-100.0,
                    bias=c50_t,
                )
            for qc in range(QC):
                sl = slice(qc * Q, (qc + 1) * Q)
                nc.scalar.activation(
                    out=chain[:, sl], in_=chain[:, sl], func=AF.Exp,
                    scale=exp_scale,
                )
                nc.vector.scalar_tensor_tensor(
                    out=chain[:, sl],
                    in0=chain[:, sl],
                    scalar=e_thresh,
                    in1=chain[:, sl],
                    op0=OP.is_lt,
                    op1=OP.mult,
                    accum_out=rs[:, qc : qc + 1],
                )
            nc.vector.tensor_scalar(
                out=neg_inv,
                in0=rs,
                scalar1=-1.0,
                scalar2=-BP_EPS,
                op0=OP.mult,
                op1=OP.add,
            )
            nc.vector.reciprocal(out=neg_inv, in_=neg_inv)
            for qc in range(QC):
                sl = slice(qc * Q, (qc + 1) * Q)
                nc.scalar.activation(
                    out=chain[:, sl],
                    in_=chain[:, sl],
                    func=AF.Ln,
                    scale=neg_inv[:, qc : qc + 1],
                    bias=1.0,
                )
                nc.scalar.activation(
                    out=chain[:, sl],
                    in_=chain[:, sl],
                    func=AF.Exp,
                    scale=float(N_SAMPLES),
                )
            mp = chain

            dve_abs = smalls.tile([P, 2], fp32)
            pool_abs = smalls.tile([P, 2], fp32)
            nc.vector.tensor_copy(out=dve_abs[:, 0:1], in_=rand_sb[0][:, 0:1])
            nc.vector.tensor_copy(out=dve_abs[:, 1:2], in_=rand_sb[1][:, 0:1])
            nc.gpsimd.tensor_copy(out=pool_abs[:, 0:1], in_=rand_sb[0][:, 0:1])
            nc.gpsimd.tensor_copy(out=pool_abs[:, 1:2], in_=rand_sb[1][:, 0:1])

            work = [
                maps.tile([P, FMAP], fp32, tag="maps", name=f"work{h}")
                for h in range(2)
            ]
            for h in range(2):
                sub_eng = nc.vector if h == 0 else nc.gpsimd
                for qc in range(QC):
                    sl = slice(qc * Q, (qc + 1) * Q)
                    sub_eng.tensor_tensor(
                        out=work[h][:, sl],
                        in0=mp[:, sl],
                        in1=rand_sb[h][:, sl],
                        op=OP.subtract,
                    )
                    nc.scalar.activation(
                        out=rand_sb[h][:, sl],
                        in_=work[h][:, sl],
                        func=AF.Sigmoid,
                        scale=100.0,
                    )
                    mul_eng = nc.vector if h == 0 else nc.gpsimd
                    mul_eng.tensor_tensor(
                        out=work[h][:, sl],
                        in0=smask[:, sl],
                        in1=rand_sb[h][:, sl],
                        op=OP.mult,
                    )
                    nc.sync.dma_start(
                        out=out_r[h, :, qc, :],
                        in_=work[h][:, sl],
                    )

    return nc


def _build(layer_id: int, fast: bool):
    """Build the per-core Bass program (same NEFF on all 8 cores)."""
    import concourse.bass as bass
    import concourse.tile as tile
    import concourse.mybir as mybir

    nc = bass.Bass("TRN2", target_bir_lowering=False)
    if fast:
        nc = _build_fast(nc, tile, mybir)
    else:
        nc = _build_full(nc, tile, mybir, layer_id)
    return _legalize_waits(nc)


def _get_nc(layer_id: int, fast: bool):
    key = (int(layer_id), bool(fast))
    if key not in _BUILD_CACHE:
        _BUILD_CACHE[key] = _build(*key)
    return _BUILD_CACHE[key]


def _fast_path_ok(tgt_mask, w_all, b_all, layer_id):
    """Host-side guard: the fast kernel assumes every attn value stays above
    the 0.5 threshold with margin (so bp==0, masked_prob==1, and the smask
    tail stays < 2e-3). Verify on the actual inputs with a cheap numpy pass.
    (layer_id only scales the Boltzmann exponent, which is inert when bp==0,
    so it does not affect fast-path validity.)"""
    del layer_id
    x = tgt_mask.astype(np.float32)
    mu = x.mean(-1, keepdims=True)
    var = x.var(-1, keepdims=True)
    xn = (x - mu) / np.sqrt(var + LN_EPS)
    h = np.maximum(xn @ w_all[0] + b_all[0], 0.0)
    h = np.maximum(h @ w_all[1] + b_all[1], 0.0)
    me = h @ w_all[2] + b_all[2]
    zmin = np.inf
    for b in range(me.shape[0]):
        s = (me[b] @ me[b].T) / np.float32(math.sqrt(D))
        zmin = min(zmin, float(s.min()))
    return zmin > 0.25


def _run(
    tgt_mask,
    ln_w,
    ln_b,
    w1,
    b1,
    w2,
    b2,
    w3,
    b3,
    rand,
    layer_id,
    trace=False,
    force_path=None,
):
    from concourse.bass_utils import run_bass_kernel_spmd

    tgt_mask = np.asarray(tgt_mask, np.float32)
    ln_w = np.asarray(ln_w, np.float32)
    ln_b = np.asarray(ln_b, np.float32)
    w1 = np.asarray(w1, np.float32)
    b1 = np.asarray(b1, np.float32)
    w2 = np.asarray(w2, np.float32)
    b2 = np.asarray(b2, np.float32)
    w3 = np.asarray(w3, np.float32)
    b3 = np.asarray(b3, np.float32)
    rand = np.asarray(rand, np.float32)
    lid = int(np.asarray(layer_id))

    # Fold the layernorm affine params into layer 1: LN(x)*g + c then @w1+b1
    # == LN(x) @ (g[:,None]*w1) + (c@w1 + b1).
    w1f = (ln_w[:, None] * w1).astype(np.float32)
    b1f = (ln_b @ w1 + b1).astype(np.float32)
    w_all = np.ascontiguousarray(np.stack([w1f, w2, w3]), np.float32)
    b_all = np.ascontiguousarray(np.stack([b1f, b2, b3]), np.float32)

    if force_path is None:
        fast = _fast_path_ok(tgt_mask, w_all, b_all, lid)
    else:
        fast = force_path == "fast"
    nc = _get_nc(lid, fast)

    rand_dev = rand.astype(np.float16) if fast else rand

    in_maps = []
    for c in range(N_CORES):
        b = c // (N_CORES // B)
        in_maps.append(
            {
                "x": np.ascontiguousarray(tgt_mask[b]),
                "w": w_all,
                "b": b_all,
                "rand": np.ascontiguousarray(
                    rand_dev[c * HEADS_PER_CORE : (c + 1) * HEADS_PER_CORE]
                ),
            }
        )

    res = run_bass_kernel_spmd(
        nc, in_maps, core_ids=list(range(N_CORES)), trace=trace
    )
    out = np.concatenate(
        [np.asarray(res.results[c]["out"]) for c in range(N_CORES)], axis=0
    )
    return out.astype(np.float32), res


def kernel(**inputs):
    out, _ = _run(**inputs)
    return out


# revision 38
# speedup vs baseline: 1.1224x; 1.0772x over previous
"""Boltzformer decoder mask kernel for Trainium2 (8 NeuronCores, SPMD).

Full-input contract: kernel(**inputs) takes the unsharded tensors from
setup_inputs() and returns the full [16, 1024, 1024] float32 output.

Sharding: data-parallel over the B*H=16 leading dim. Core c handles batch
c//4 and the two head-slices (2c, 2c+1). The attention map is identical
across heads within a batch, so each core computes LN -> MLP -> me@me^T ->
sigmoid chain once, and only the rand-dependent tail twice.

Two compiled variants:
- fast: valid when every score is comfortably above the 0.5 attn threshold
  (checked on the host against the actual inputs). Then bp == 0 and
  masked_prob == 1 exactly, so the Boltzmann-normalize chain drops out.
  Key design points (all verified against the reference):
  * Matmuls stay exact fp32: the reference output is 1 - sigmoid_fp32(z),
    whose catastrophic cancellation quantizes values to multiples of
    2^-23; the surviving nonzeros are a few hundred elements of 1-14 ulp,
    so ONE quantization-boundary flip costs ~2.5% of the output norm.
    attn must match jax to ~1e-7 (fp32r's ~2^-16 noise would flip many).
    The (t+1)-1 fixup on the device reproduces the reference quantization
    exactly and also snaps away sub-half-ulp hardware LUT noise.
  * The [1024,1024] attention map is symmetric (scores = me@me^T):
    scores/sigmoid/exp run only on upper-triangle bands; lower blocks are
    mirrored with PE 128x128 bf16 transposes (ACT is the scarce engine).
  * rand streams in as fp16 (host-rounded, halves the biggest input);
    boltz and smask are bf16 (quantized smask values are bf16-exact);
    out is bf16, upcast on the host -- 45% less HBM traffic total.
  * Scores PSUM pieces are 512-aligned: a matmul output must not cross a
    2KB PSUM bank boundary.
  * PSUM pools cycle slots; sigma(qc) must be EMITTED before
    scores_alloc(qc+3) or the slot WAR misses not-yet-emitted readers.
- full: the general computation, bit-conservative fp32 throughout (used
  if the guard ever fails).
"""

import math

import numpy as np

B = 2
Q = 1024
D = 256
NUM_HEADS = 8
N_CORES = 8
HEADS_PER_CORE = 2
THRESHOLD = 0.5
N_SAMPLES = int(Q * 0.1)  # 102
LN_EPS = 1e-4
BP_EPS = 1e-6
P = 128  # SBUF partitions
QC = Q // P  # 8 row-chunks per map
FMAP = QC * Q  # [1024,1024] map stored as [128, 8192]

_BUILD_CACHE = {}


def _legalize_waits(nc):
    """TRN2 instruction structs carry only ONE inline sync-wait slot (fp32
    self-loading matmuls, activations, DVE tensor ops, DMA descriptors
    alike). Tile attaches multi-waits; legalize by hoisting the excess waits
    onto standalone same-engine NoOps right before the instruction (the
    raw-bass "wait_ge then op" idiom). Walrus partitions blocks by engine
    preserving order, so a NoOp inserted directly before stays ahead in that
    engine's queue -- semantics are preserved exactly."""
    import concourse.mybir as mybir
    import bass_rust

    skip = ("InstDmaTransposeAnt", "InstTriggerDma")
    for blk in nc.m.functions[0].blocks:
        out_list = []
        for ins in blk.instructions:
            si = getattr(ins, "sync_info", None)
            eng = getattr(ins, "engine", None)
            if (
                si is not None
                and eng is not None
                and type(ins).__name__ not in skip
                and len(si.on_wait) > 1
            ):
                waits = list(si.on_wait)
                for j, w in enumerate(waits[:-1]):
                    nop = mybir.InstNoOp(name=f"{ins.name}-ws{j}", ins=[], outs=[])
                    nop.engine = eng
                    nop.sync_info = bass_rust.SyncInfo(on_wait=[w], on_update=[])
                    out_list.append(nop)
                si.on_wait = [waits[-1]]
            out_list.append(ins)
        blk.instructions = out_list
    return nc


def _build_fast(nc, tile, mybir):
    """Fast-path per-core program.

    Engine budget per core (cost-model): DMA 10.2 MiB ~= 28.7us pooled;
    PE (exact-fp32 matmuls: 3-layer MLP 20.5us + triangular scores 7.7us)
    and ACT (2 boltz map passes + triangular sigmoid/exp) are the pacing
    engines. The MLP runs token-block-0 chain first, then the first
    score-band pieces (cols < 512, which only need meT block 0), then the
    token-block-1 chain -- so the sigmoid/exp/out stream starts while the
    second half of the MLP still runs. Mults on DVE (bf16 2x mode), h1
    rows 0-5 on Pool (SBUF-only: Pool cannot touch PSUM).
    """
    import contextlib

    fp32 = mybir.dt.float32
    fp32r = mybir.dt.float32r
    fp16 = mybir.dt.float16
    bf16 = mybir.dt.bfloat16
    AF = mybir.ActivationFunctionType
    OP = mybir.AluOpType

    x_d = nc.dram_tensor("x", [Q, D], fp32, kind="ExternalInput")
    w_d = nc.dram_tensor("w", [3, D, D], fp32, kind="ExternalInput")
    b_d = nc.dram_tensor("b", [3, D], fp32, kind="ExternalInput")
    rand_d = nc.dram_tensor("rand", [HEADS_PER_CORE, Q, Q], fp16, kind="ExternalInput")
    out_d = nc.dram_tensor("out", [HEADS_PER_CORE, Q, Q], bf16, kind="ExternalOutput")

    with tile.TileContext(nc) as tc:
        ctx = contextlib.ExitStack()
        with ctx:
            consts = ctx.enter_context(tc.tile_pool(name="consts", bufs=1))
            smalls = ctx.enter_context(tc.tile_pool(name="smalls", bufs=1))
            xnp = ctx.enter_context(tc.tile_pool(name="xnp", bufs=2))
            acts = ctx.enter_context(tc.tile_pool(name="acts", bufs=8))
            maps = ctx.enter_context(tc.tile_pool(name="maps", bufs=5))
            attnp = ctx.enter_context(tc.tile_pool(name="attnp", bufs=4))
            outp = ctx.enter_context(tc.tile_pool(name="outp", bufs=4))

            # ---- input DMAs (emission order == DMA priority order) ----
            # Interleave x / rand / w: x gates LN->MLP (needs to finish by
            # ~13us), the first rand slices gate ACT's boltz start (~5.4us),
            # w gates the MLP (~9.7us).
            x_sb = smalls.tile([P, QC, D], fp32)
            x_r = x_d[:, :].rearrange("(t p) d -> p t d", p=P)

            def x_dma(g):  # 2-chunk slice of x
                nc.sync.dma_start(
                    out=x_sb[:, 2 * g : 2 * g + 2, :],
                    in_=x_r[:, 2 * g : 2 * g + 2, :],
                )

            rand_sb = [
                maps.tile([P, FMAP], fp16, tag="maps", name=f"rand_sb{h}")
                for h in range(2)
            ]
            rand_r = [
                rand_d[h, :, :].rearrange("(t p) k -> p t k", p=P) for h in range(2)
            ]

            def rand_dma(h, g):  # 2-chunk slice [P, 2, 1024] of head h
                nc.sync.dma_start(
                    out=rand_sb[h].rearrange("p (t k) -> p t k", k=Q)[
                        :, 2 * g : 2 * g + 2, :
                    ],
                    in_=rand_r[h][:, 2 * g : 2 * g + 2, :],
                )

            x_dma(0)
            rand_dma(0, 0)
            x_dma(1)
            rand_dma(1, 0)
            x_dma(2)
            x_dma(3)
            w_sb = consts.tile([P, 3, 2, D], fp32)
            nc.sync.dma_start(
                out=w_sb, in_=w_d[:, :, :].rearrange("l (kc p) f -> p l kc f", p=P)
            )
            b_sb = consts.tile([P, 3, 2], fp32)
            nc.sync.dma_start(
                out=b_sb, in_=b_d[:, :].rearrange("l (c p) -> p l c", p=P)
            )
            rand_dma(0, 1)
            rand_dma(1, 1)
            rand_dma(0, 2)
            rand_dma(1, 2)
            rand_dma(0, 3)
            rand_dma(1, 3)

            identity = consts.tile([P, P], fp32)
            nc.gpsimd.memset(identity, 0.0)
            nc.gpsimd.affine_select(
                out=identity,
                in_=identity,
                compare_op=OP.not_equal,
                fill=1.0,
                base=0,
                pattern=[[-1, P]],
                channel_multiplier=1,
            )
            id_bf = consts.tile([P, P], bf16)
            nc.gpsimd.memset(id_bf, 0.0)
            nc.gpsimd.affine_select(
                out=id_bf,
                in_=id_bf,
                compare_op=OP.not_equal,
                fill=1.0,
                base=0,
                pattern=[[-1, P]],
                channel_multiplier=1,
            )

            # ---- small constants ----
            stats = smalls.tile([P, QC, 6], fp32)
            mv = smalls.tile([P, QC, 2], fp32)
            sd = smalls.tile([P, QC], fp32)
            rstd = smalls.tile([P, QC], fp32)
            eps_t = smalls.tile([P, 1], fp32)
            nc.vector.memset(eps_t, LN_EPS)
            c50_t = smalls.tile([P, 1], fp32)
            nc.vector.memset(c50_t, 50.0)
            c100_t = smalls.tile([P, 1], fp32)
            nc.vector.memset(c100_t, 100.0)

            # maps-sized tiles: rand fp16 x2, boltz bf16 x2, smask bf16
            boltz = [
                maps.tile([P, FMAP], bf16, tag="maps", name=f"boltz{h}")
                for h in range(2)
            ]
            smask = maps.tile([P, FMAP], bf16, tag="maps", name="smask")

            def boltz_op(h, g):  # one 2-chunk sigmoid over rand slice (h, g)
                sl = slice(2 * g * Q, (2 * g + 2) * Q)
                nc.scalar.activation(
                    out=boltz[h][:, sl],
                    in_=rand_sb[h][:, sl],
                    func=AF.Sigmoid,
                    scale=-100.0,
                    bias=c100_t,
                )

            def sq_op(g):  # batched sqrt(var+eps) for chunks 2g, 2g+1 on ACT
                nc.scalar.activation(
                    out=sd[:, 2 * g : 2 * g + 2],
                    in_=mv[:, 2 * g : 2 * g + 2, 1],
                    func=AF.Sqrt,
                    bias=eps_t,
                    scale=1.0,
                )

            # ---- LayerNorm: DVE stats in x-arrival order; ACT sqrts are
            # batched per 2 chunks and interleaved between boltz ops below.
            xn = [
                xnp.tile([P, QC // 2, D], fp32, tag="xn", name=f"xn{i}")
                for i in range(2)
            ]

            def ln_stats(t):
                nc.vector.bn_stats(out=stats[:, t, :], in_=x_sb[:, t, :])
                nc.vector.bn_aggr(out=mv[:, t, :], in_=stats[:, t, :])

            def ln_xn(t):
                nc.vector.reciprocal(
                    out=rstd[:, t : t + 1], in_=sd[:, t : t + 1]
                )
                nc.vector.tensor_scalar(
                    out=xn[t // 4][:, t % 4, :],
                    in0=x_sb[:, t, :],
                    scalar1=mv[:, t, 0:1],
                    scalar2=rstd[:, t : t + 1],
                    op0=OP.subtract,
                    op1=OP.mult,
                )

            for t in range(4):
                ln_stats(t)
            # ACT queue head: first boltz slice, then the batched LN sqrts
            # (so they never head-block the boltz stream), second boltz.
            boltz_op(0, 0)
            sq_op(0)
            sq_op(1)
            for t in range(4, 8):
                ln_stats(t)
            for t in range(4):
                ln_xn(t)
            boltz_op(1, 0)
            with tc.high_priority():
                sq_op(2)
                sq_op(3)
                for t in range(4, 8):
                    ln_xn(t)

            xT = [
                acts.tile([P, Q], fp32, tag="actT", name=f"xT{h}")
                for h in range(2)
            ]
            pework = ctx.enter_context(
                tc.tile_pool(name="pework", bufs=2, space="PSUM")
            )
            # PE p-state warm-up: ~6 dummy transposes so the PE is at full
            # clock when the real transposes and the MLP arrive.
            warm = smalls.tile([P, P], fp32)
            for _ in range(6):
                wps = pework.tile([P, P], fp32, tag="pe")
                nc.tensor.transpose(wps, identity, identity)
                nc.vector.tensor_copy(out=warm, in_=wps)
            if True:

                def trans_chunk(t):  # xn chunk t -> xT columns, both halves
                    for h in range(2):
                        pst = pework.tile([P, P], fp32, tag="pe")
                        nc.tensor.transpose(
                            pst, xn[t // 4][:, t % 4, h * P : (h + 1) * P], identity
                        )
                        nc.vector.tensor_copy(
                            out=xT[h][:, t * P : (t + 1) * P], in_=pst
                        )

                # ---- MLP: float32r; emitted per (layer, token-block) so
                # the PE queue reaches layer L+1 of token-block 0 without
                # head-blocking on late chunk-4..7 transposes. Biases: fc0
                # on DVE, fc1 on Pool.
                y = [
                    [
                        acts.tile([P, Q], fp32, tag="actT", name=f"y{layer}T{f2}")
                        for f2 in range(2)
                    ]
                    for layer in range(3)
                ]

                def mlp_lrc(layer, rc):
                    cur = xT if layer == 0 else y[layer - 1]
                    if True:
                        for fc in range(2):
                            ps = pework.tile([P, 512], fp32, tag="pe")
                            for kc in range(2):
                                nc.tensor.matmul(
                                    ps,
                                    lhsT=w_sb[
                                        :, layer, kc, fc * P : (fc + 1) * P
                                    ],
                                    rhs=cur[kc][
                                        :, rc * 512 : (rc + 1) * 512
                                    ],
                                    start=(kc == 0),
                                    stop=(kc == 1),
                                )
                            if layer == 0 and rc == 0:
                                nc.scalar.activation(
                                    out=y[0][fc][:, 0:512],
                                    in_=ps,
                                    func=AF.Relu,
                                    bias=b_sb[:, 0, fc : fc + 1],
                                    scale=1.0,
                                )
                                continue
                            eng = nc.vector
                            if layer < 2:
                                eng.tensor_scalar(
                                    out=y[layer][fc][:, rc * 512 : (rc + 1) * 512],
                                    in0=ps,
                                    scalar1=b_sb[:, layer, fc : fc + 1],
                                    scalar2=0.0,
                                    op0=OP.add,
                                    op1=OP.max,
                                )
                            else:
                                eng.tensor_scalar(
                                    out=y[layer][fc][:, rc * 512 : (rc + 1) * 512],
                                    in0=ps,
                                    scalar1=b_sb[:, layer, fc : fc + 1],
                                    scalar2=None,
                                    op0=OP.add,
                                )

                for t in range(4):
                    trans_chunk(t)
                mlp_lrc(0, 0)
                for t in range(4, 8):
                    trans_chunk(t)
                mlp_lrc(1, 0)
                mlp_lrc(2, 0)
            meT = y[2]  # [2][128, 1024] feature-major me^T

            out_r = out_d[:, :, :].rearrange("h (t p) k -> p h t k", p=P)
            spsum = ctx.enter_context(
                tc.tile_pool(name="spsum", bufs=3, space="PSUM")
            )

            # ---- triangular scores -> attn -> smask (upper bands only) ----
            # Band qc covers columns [qc*128, 1024). Piece boundaries are
            # aligned to 512 so piece 1 only needs meT token-block 0.
            band_ps = {}
            attn_bands = {}

            def band_pieces(qc):
                lo = qc * P
                if lo < 512:
                    return [(lo, 512), (512, Q)]
                return [(lo, Q)]

            def scores_alloc(qc):
                # ps spans the full row so each piece's matmul output stays
                # inside one 2KB PSUM bank (a matmul output must not cross
                # a bank boundary; only cols [qc*128, 1024) are written).
                band_ps[qc] = spsum.tile([P, Q], fp32, tag="sps", name=f"sps{qc}")
                attn_bands[qc] = attnp.tile(
                    [P, Q], fp32, tag="attn", name=f"attn{qc}"
                )

            def scores_mm(qc, piece):
                lo = qc * P
                ps = band_ps[qc]
                c0, c1 = piece
                for kc in range(2):
                    nc.tensor.matmul(
                        ps[:, c0:c1],
                        lhsT=meT[kc][:, lo : lo + P],
                        rhs=meT[kc][:, c0:c1],
                        start=(kc == 0),
                        stop=(kc == 1),
                    )

            def sig_exp(qc, piece=None):
                # sigmoid (PSUM->attn fp32) then exp -> smask band, bf16.
                # Exp LUT keeps RELATIVE accuracy at the ~1e-6 magnitudes
                # that dominate the output norm (sigmoid LUT would not).
                lo = qc * P
                pieces = [(lo, Q)] if piece is None else [piece]
                ps = band_ps[qc]
                attn_c = attn_bands[qc]
                for c0, c1 in pieces:
                    nc.scalar.activation(
                        out=attn_c[:, c0:c1],
                        in_=ps[:, c0:c1],
                        func=AF.Sigmoid,
                        scale=1.0 / math.sqrt(D),
                    )
                    # t = exp(-100*(attn-0.5)), in place (fp32)
                    nc.scalar.activation(
                        out=attn_c[:, c0:c1],
                        in_=attn_c[:, c0:c1],
                        func=AF.Exp,
                        scale=-100.0,
                        bias=c50_t,
                    )
                    # smask = (t+1)-1 in fp32, rounded to bf16 on write:
                    # reproduces the reference's catastrophic-cancellation
                    # quantization of 1-sigmoid(z) (multiples of 2^-23 --
                    # 8.4% of the output norm IS this quantization), and
                    # those grid values are exact in bf16. Also snaps away
                    # any hardware Exp-LUT noise below half a grid step.
                    qeng = nc.vector
                    qeng.tensor_scalar(
                        out=smask[:, qc * Q + c0 : qc * Q + c1],
                        in0=attn_c[:, c0:c1],
                        scalar1=1.0,
                        scalar2=1.0,
                        op0=OP.add,
                        op1=OP.subtract,
                    )

            def mirror_group(b, part=None):
                # reflect band b's blocks (a, b) for a > b: PE transpose of
                # each bf16 [128,128] block + PSUM->SBUF copy on DVE.
                # part='lo' mirrors only rows a<=3 (available from band b's
                # first 512 cols), 'hi' rows a>=4; None = all.
                rows = range(b + 1, QC)
                if part == 'lo':
                    rows = range(b + 1, 4)
                elif part == 'hi':
                    rows = range(max(b + 1, 4), QC)
                for a in rows:
                    pst = pework.tile([P, P], bf16, tag="pe")
                    nc.tensor.transpose(
                        pst, smask[:, b * Q + a * P : b * Q + (a + 1) * P], id_bf
                    )
                    nc.vector.tensor_copy(
                        out=smask[:, a * Q + b * P : a * Q + (b + 1) * P],
                        in_=pst,
                    )

            out_tiles = {}

            def mult_out(qc, piece=None):
                if qc not in out_tiles:
                    out_tiles[qc] = outp.tile(
                        [P, 2, Q], bf16, tag="out", name=f"out{qc}"
                    )
                out_t = out_tiles[qc]
                c0, c1 = piece if piece is not None else (0, Q)
                for h in range(2):
                    eng = nc.gpsimd if (h == 1 and qc < 4) else nc.vector
                    eng.tensor_tensor(
                        out=out_t[:, h, c0:c1],
                        in0=smask[:, qc * Q + c0 : qc * Q + c1],
                        in1=boltz[h][:, qc * Q + c0 : qc * Q + c1],
                        op=OP.mult,
                    )
                nc.sync.dma_start(
                    out=out_r[:, :, qc, c0:c1],
                    in_=out_t[:, :, c0:c1],
                )

            # ---- tail: the interleave order below IS each engine's queue
            # order (ACT: boltz/sigmoid/exp; PE: scores then mirrors; DVE:
            # mirror copies + mults). Tuned so ACT never head-blocks and the
            # out-DMA stream starts as early as possible.
            scores_alloc(0)
            scores_mm(0, (0, 512))
            scores_alloc(1)
            scores_mm(1, (128, 512))
            scores_alloc(2)
            scores_mm(2, (256, 512))
            mlp_lrc(0, 1)
            mlp_lrc(1, 1)
            mlp_lrc(2, 1)
            boltz_op(0, 1)
            boltz_op(1, 1)
            boltz_op(0, 2)
            boltz_op(1, 2)
            boltz_op(0, 3)
            boltz_op(1, 3)
            sig_exp(0, piece=(0, 512))
            mult_out(0, piece=(0, 512))
            sig_exp(1, piece=(128, 512))
            mirror_group(0, part='lo')
            sig_exp(2, piece=(256, 512))
            mirror_group(1, part='lo')
            mirror_group(2, part='lo')
            mult_out(1, piece=(0, 512))
            mult_out(2, piece=(0, 512))
            scores_mm(0, (512, Q))
            sig_exp(0, piece=(512, Q))
            mult_out(0, piece=(512, Q))
            scores_mm(1, (512, Q))
            sig_exp(1, piece=(512, Q))
            mirror_group(0, part='hi')
            mult_out(1, piece=(512, Q))
            scores_mm(2, (512, Q))
            sig_exp(2, piece=(512, Q))
            mirror_group(1, part='hi')
            mult_out(2, piece=(512, Q))
            scores_alloc(3)
            scores_mm(3, (384, 512))
            scores_mm(3, (512, Q))
            sig_exp(3)
            mirror_group(2, part='hi')
            scores_alloc(4)
            scores_mm(4, (512, Q))
            sig_exp(4)
            mirror_group(3)
            mult_out(3)
            scores_alloc(5)
            scores_mm(5, (640, Q))
            sig_exp(5)
            mirror_group(4)
            mult_out(4)
            scores_alloc(6)
            scores_mm(6, (768, Q))
            sig_exp(6)
            mirror_group(5)
            mult_out(5)
            scores_alloc(7)
            scores_mm(7, (896, Q))
            sig_exp(7)
            mirror_group(6)
            mult_out(6, piece=(0, 512))
            mult_out(7, piece=(0, 512))
            mult_out(6, piece=(512, Q))
            mult_out(7, piece=(512, Q))

    return nc


def _build_full(nc, tile, mybir, layer_id):
    """General path: full Boltzmann chain, conservative fp32 (bit-matched
    to the reference where possible). Same code as the original baseline."""
    import contextlib

    fp32 = mybir.dt.float32
    AF = mybir.ActivationFunctionType
    OP = mybir.AluOpType

    exp_scale = 2.0 + float(layer_id)  # attn / temp == attn * (2 + layer_id)

    x_d = nc.dram_tensor("x", [Q, D], fp32, kind="ExternalInput")
    w_d = nc.dram_tensor("w", [3, D, D], fp32, kind="ExternalInput")
    b_d = nc.dram_tensor("b", [3, D], fp32, kind="ExternalInput")
    rand_d = nc.dram_tensor("rand", [HEADS_PER_CORE, Q, Q], fp32, kind="ExternalInput")
    out_d = nc.dram_tensor("out", [HEADS_PER_CORE, Q, Q], fp32, kind="ExternalOutput")

    with tile.TileContext(nc) as tc:
        ctx = contextlib.ExitStack()
        with ctx:
            consts = ctx.enter_context(tc.tile_pool(name="consts", bufs=1))
            smalls = ctx.enter_context(tc.tile_pool(name="smalls", bufs=1))
            acts = ctx.enter_context(tc.tile_pool(name="acts", bufs=4))
            maps = ctx.enter_context(tc.tile_pool(name="maps", bufs=5))

            x_sb = smalls.tile([P, QC, D], fp32)
            x_r = x_d[:, :].rearrange("(t p) d -> p t d", p=P)
            for t in range(QC):
                nc.sync.dma_start(out=x_sb[:, t, :], in_=x_r[:, t, :])
            w_sb = consts.tile([P, 3, 2, D], fp32)
            nc.sync.dma_start(
                out=w_sb, in_=w_d[:, :, :].rearrange("l (kc p) f -> p l kc f", p=P)
            )
            b_sb = consts.tile([P, 3, 2], fp32)
            nc.sync.dma_start(
                out=b_sb, in_=b_d[:, :].rearrange("l (c p) -> p l c", p=P)
            )
            rand_sb = [
                maps.tile([P, FMAP], fp32, tag="maps", name=f"rand_sb{h}")
                for h in range(2)
            ]
            for h in range(2):
                nc.sync.dma_start(
                    out=rand_sb[h].rearrange("p (t k) -> p t k", k=Q),
                    in_=rand_d[h, :, :].rearrange("(t p) k -> p t k", p=P),
                )

            identity = consts.tile([P, P], fp32)
            nc.gpsimd.memset(identity, 0.0)
            nc.gpsimd.affine_select(
                out=identity,
                in_=identity,
                compare_op=OP.not_equal,
                fill=1.0,
                base=0,
                pattern=[[-1, P]],
                channel_multiplier=1,
            )

            stats = smalls.tile([P, QC, 6], fp32)
            mv = smalls.tile([P, QC, 2], fp32)
            sd = smalls.tile([P, QC], fp32)
            rstd = smalls.tile([P, QC], fp32)
            eps_t = smalls.tile([P, 1], fp32)
            nc.vector.memset(eps_t, LN_EPS)
            c50_t = smalls.tile([P, 1], fp32)
            nc.vector.memset(c50_t, 50.0)
            c100_t = smalls.tile([P, 1], fp32)
            nc.vector.memset(c100_t, 100.0)
            xn = [
                acts.tile([P, QC // 2, D], fp32, tag="actT", name=f"xn{i}")
                for i in range(2)
            ]
            for t in range(QC):
                nc.vector.bn_stats(out=stats[:, t, :], in_=x_sb[:, t, :])
                nc.vector.bn_aggr(out=mv[:, t, :], in_=stats[:, t, :])
                nc.scalar.activation(
                    out=sd[:, t : t + 1],
                    in_=mv[:, t, 1:2],
                    func=AF.Sqrt,
                    bias=eps_t,
                    scale=1.0,
                )
                nc.vector.reciprocal(
                    out=rstd[:, t : t + 1], in_=sd[:, t : t + 1]
                )
                nc.vector.tensor_scalar(
                    out=xn[t // 4][:, t % 4, :],
                    in0=x_sb[:, t, :],
                    scalar1=mv[:, t, 0:1],
                    scalar2=rstd[:, t : t + 1],
                    op0=OP.subtract,
                    op1=OP.mult,
                )

            b_abs = smalls.tile([P, 1], fp32)
            nc.vector.tensor_copy(out=b_abs, in_=b_sb[:, 0, 0:1])

            xT = [acts.tile([P, Q], fp32, tag="actT", name=f"xT{h}") for h in range(2)]
            with tc.tile_pool(name="tpsum", bufs=4, space="PSUM") as tpsum, \
                 tc.tile_pool(name="mlpp", bufs=4, space="PSUM") as mlpp:
                for t in range(QC):
                    for h in range(2):
                        pst = tpsum.tile([P, P], fp32)
                        nc.tensor.transpose(
                            pst, xn[t // 4][:, t % 4, h * P : (h + 1) * P], identity
                        )
                        if (t * 2 + h) % 2 == 0:
                            nc.scalar.copy(
                                out=xT[h][:, t * P : (t + 1) * P], in_=pst
                            )
                        else:
                            nc.vector.tensor_copy(
                                out=xT[h][:, t * P : (t + 1) * P], in_=pst
                            )

                cur = xT
                for layer in range(3):
                    nxt = [
                        acts.tile([P, Q], fp32, tag="actT", name=f"y{layer}T{f2}")
                        for f2 in range(2)
                    ]
                    for fc in range(2):
                        for rc in range(2):
                            ps = mlpp.tile([P, 512], fp32)
                            for kc in range(2):
                                nc.tensor.matmul(
                                    ps,
                                    lhsT=w_sb[:, layer, kc, fc * P : (fc + 1) * P],
                                    rhs=cur[kc][:, rc * 512 : (rc + 1) * 512],
                                    start=(kc == 0),
                                    stop=(kc == 1),
                                )
                            if layer < 2:
                                nc.vector.tensor_scalar(
                                    out=nxt[fc][:, rc * 512 : (rc + 1) * 512],
                                    in0=ps,
                                    scalar1=b_sb[:, layer, fc : fc + 1],
                                    scalar2=0.0,
                                    op0=OP.add,
                                    op1=OP.max,
                                )
                            else:
                                nc.vector.tensor_scalar(
                                    out=nxt[fc][:, rc * 512 : (rc + 1) * 512],
                                    in0=ps,
                                    scalar1=b_sb[:, layer, fc : fc + 1],
                                    scalar2=None,
                                    op0=OP.add,
                                )
                    cur = nxt
            meT = cur

            smask = maps.tile([P, FMAP], fp32, tag="maps")
            out_r = out_d[:, :, :].rearrange("h (t p) k -> h p t k", p=P)
            spsum = ctx.enter_context(
                tc.tile_pool(name="spsum", bufs=4, space="PSUM")
            )

            chain = maps.tile([P, FMAP], fp32, tag="maps")
            attn = chain
            for qc in range(QC):
                ps = spsum.tile([P, Q], fp32)
                for nh in range(2):
                    for kc in range(2):
                        nc.tensor.matmul(
                            ps[:, nh * 512 : (nh + 1) * 512],
                            lhsT=meT[kc][:, qc * P : (qc + 1) * P],
                            rhs=meT[kc][:, nh * 512 : (nh + 1) * 512],
                            start=(kc == 0),
                            stop=(kc == 1),
                        )
                nc.scalar.activation(
                    out=attn[:, qc * Q : (qc + 1) * Q],
                    in_=ps,
                    func=AF.Sigmoid,
                    scale=1.0 / math.sqrt(D),
                )

            rs = smalls.tile([P, QC], fp32)
            neg_inv = smalls.tile([P, QC], fp32)
            e_thresh = float(np.exp(np.float32(THRESHOLD * exp_scale)))
            for qc in range(QC):
                sl = slice(qc * Q, (qc + 1) * Q)
                nc.scalar.activation(
                    out=smask[:, sl],
                    in_=attn[:, sl],
                    func=AF.Sigmoid,
                    scale=-100.0,
                    bias=c50_t,
                )
            for qc in range(QC):
                sl = slice(qc * Q, (qc + 1) * Q)
                nc.scalar.activation(
                    out=chain[:, sl], in_=chain[:, sl], func=AF.Exp,
                    scale=exp_scale,
                )
                nc.vector.scalar_tensor_tensor(
                    out=chain[:, sl],
                    in0=chain[:, sl],
                    scalar=e_thresh,
                    in1=chain[:, sl],
                    op0=OP.is_lt,
                    op1=OP.mult,
                    accum_out=rs[:, qc : qc + 1],
                )
            nc.vector.tensor_scalar(
                out=neg_inv,
                in0=rs,
                scalar1=-1.0,
                scalar2=-BP_EPS,
                op0=OP.mult,
                op1=OP.add,
            )
            nc.vector.reciprocal(out=neg_inv, in_=neg_inv)
            for qc in range(QC):
                sl = slice(qc * Q, (qc + 1) * Q)
                nc.scalar.activation(
                    out=chain[:, sl],
                    in_=chain[:, sl],
                    func=AF.Ln,
                    scale=neg_inv[:, qc : qc + 1],
                    bias=1.0,
                )
                nc.scalar.activation(
                    out=chain[:, sl],
                    in_=chain[:, sl],
                    func=AF.Exp,
                    scale=float(N_SAMPLES),
                )
            mp = chain

            dve_abs = smalls.tile([P, 2], fp32)
            pool_abs = smalls.tile([P, 2], fp32)
            nc.vector.tensor_copy(out=dve_abs[:, 0:1], in_=rand_sb[0][:, 0:1])
            nc.vector.tensor_copy(out=dve_abs[:, 1:2], in_=rand_sb[1][:, 0:1])
            nc.gpsimd.tensor_copy(out=pool_abs[:, 0:1], in_=rand_sb[0][:, 0:1])
            nc.gpsimd.tensor_copy(out=pool_abs[:, 1:2], in_=rand_sb[1][:, 0:1])

            work = [
                maps.tile([P, FMAP], fp32, tag="maps", name=f"work{h}")
                for h in range(2)
            ]
            for h in range(2):
                sub_eng = nc.vector if h == 0 else nc.gpsimd
                for qc in range(QC):
                    sl = slice(qc * Q, (qc + 1) * Q)
                    sub_eng.tensor_tensor(
                        out=work[h][:, sl],
                        in0=mp[:, sl],
                        in1=rand_sb[h][:, sl],
                        op=OP.subtract,
                    )
                    nc.scalar.activation(
                        out=rand_sb[h][:, sl],
                        in_=work[h][:, sl],
                        func=AF.Sigmoid,
                        scale=100.0,
                    )
                    mul_eng = nc.vector if h == 0 else nc.gpsimd
                    mul_eng.tensor_tensor(
                        out=work[h][:, sl],
                        in0=smask[:, sl],
                        in1=rand_sb[h][:, sl],
                        op=OP.mult,
                    )
                    nc.sync.dma_start(
                        out=out_r[h, :, qc, :],
                        in_=work[h][:, sl],
                    )

    return nc


def _build(layer_id: int, fast: bool):
    """Build the per-core Bass program (same NEFF on all 8 cores)."""
    import concourse.bass as bass
    import concourse.tile as tile
    import concourse.mybir as mybir

    nc = bass.Bass("TRN2", target_bir_lowering=False)
    if fast:
        nc = _build_fast(nc, tile, mybir)
    else:
        nc = _build_full(nc, tile, mybir, layer_id)
    return _legalize_waits(nc)


def _get_nc(layer_id: int, fast: bool):
    key = (int(layer_id), bool(fast))
    if key not in _BUILD_CACHE:
        _BUILD_CACHE[key] = _build(*key)
    return _BUILD_CACHE[key]


def _fast_path_ok(tgt_mask, w_all, b_all, layer_id):
    """Host-side guard: the fast kernel assumes every attn value stays above
    the 0.5 threshold with margin (so bp==0, masked_prob==1, and the smask
    tail stays < 2e-3). Verify on the actual inputs with a cheap numpy pass.
    (layer_id only scales the Boltzmann exponent, which is inert when bp==0,
    so it does not affect fast-path validity.)"""
    del layer_id
    x = tgt_mask.astype(np.float32)
    mu = x.mean(-1, keepdims=True)
    var = x.var(-1, keepdims=True)
    xn = (x - mu) / np.sqrt(var + LN_EPS)
    h = np.maximum(xn @ w_all[0] + b_all[0], 0.0)
    h = np.maximum(h @ w_all[1] + b_all[1], 0.0)
    me = h @ w_all[2] + b_all[2]
    zmin = np.inf
    for b in range(me.shape[0]):
        s = (me[b] @ me[b].T) / np.float32(math.sqrt(D))
        zmin = min(zmin, float(s.min()))
    return zmin > 0.25


def _run(
    tgt_mask,
    ln_w,
    ln_b,
    w1,
    b1,
    w2,
    b2,
    w3,
    b3,
    rand,
    layer_id,
    trace=False,
    force_path=None,
):
    from concourse.bass_utils import run_bass_kernel_spmd

    tgt_mask = np.asarray(tgt_mask, np.float32)
    ln_w = np.asarray(ln_w, np.float32)
    ln_b = np.asarray(ln_b, np.float32)
    w1 = np.asarray(w1, np.float32)
    b1 = np.asarray(b1, np.float32)
    w2 = np.asarray(w2, np.float32)
    b2 = np.asarray(b2, np.float32)
    w3 = np.asarray(w3, np.float32)
    b3 = np.asarray(b3, np.float32)
    rand = np.asarray(rand, np.float32)
    lid = int(np.asarray(layer_id))

    # Fold the layernorm affine params into layer 1: LN(x)*g + c then @w1+b1
    # == LN(x) @ (g[:,None]*w1) + (c@w1 + b1).
    w1f = (ln_w[:, None] * w1).astype(np.float32)
    b1f = (ln_b @ w1 + b1).astype(np.float32)
    w_all = np.ascontiguousarray(np.stack([w1f, w2, w3]), np.float32)
    b_all = np.ascontiguousarray(np.stack([b1f, b2, b3]), np.float32)

    if force_path is None:
        fast = _fast_path_ok(tgt_mask, w_all, b_all, lid)
    else:
        fast = force_path == "fast"
    nc = _get_nc(lid, fast)

    rand_dev = rand.astype(np.float16) if fast else rand

    in_maps = []
    for c in range(N_CORES):
        b = c // (N_CORES // B)
        in_maps.append(
            {
                "x": np.ascontiguousarray(tgt_mask[b]),
                "w": w_all,
                "b": b_all,
                "rand": np.ascontiguousarray(
                    rand_dev[c * HEADS_PER_CORE : (c + 1) * HEADS_PER_CORE]
                ),
            }
        )

    res = run_bass_kernel_spmd(
        nc, in_maps, core_ids=list(range(N_CORES)), trace=trace
    )
    out = np.concatenate(
        [np.asarray(res.results[c]["out"]) for c in range(N_CORES)], axis=0
    )
    return out.astype(np.float32), res


def kernel(**inputs):
    out, _ = _run(**inputs)
    return out
